# revision 19
# baseline (speedup 1.0000x reference)
"""MoE top-2 (Switch-style) expert-parallel kernel for Trainium2, 8 NeuronCores.

Model dims (hardcoded from the problem spec):
  x:[B=8,S=512,D=512], w_gate:[D,E=8], W1:[E,D,H=1024], b1:[E,H],
  W2:[E,H,D], b2:[E,D], top-k K=2, mask:[B,S] in {0,1}.

Strategy (expert-parallel, matching the sharding hint):
  - Host computes the gating (logits -> top-2 -> softmax -> dense gates,
    masked) and dispatches: for each expert e, gather the tokens with a
    non-zero gate for e into a capacity-C buffer, transposed to [D, C]
    so the device never has to transpose activations.
  - Each of the 8 cores runs one expert's 2-layer MLP on its [D, C]
    token block, entirely in "transposed" layout:
        hT[H,C]   = relu(W1e^T-form matmul: lhsT=W1e[D,H], rhs=xeT[D,C]) + b1
        outT[D,C] = (lhsT=W2e[H,D], rhs=hT[H,C]) + b2
    fp32 data, fp32r matmuls (full PE rate at N>=256).
  - Host combines: y = x + sum_e gate_e * outT_e^T scattered back to the
    token positions (exactly equal to the dense reference formulation,
    since non-top-2 gates are exactly zero).

Codegen quirk this kernel works around: the walrus build here allows only
ONE sync-wait on Matmult/Activation/DMA instruction structs and ~8 on the
kernel-tail Drain. Hence: one DMA per input tensor (few HWDGE lanes),
tiny "absorber" ops that soak up DMA-completion waits, and the store on
the SWDGE (gpsimd) queue.
"""

import math

import numpy as np

B, S, D, H, E, TOPK = 8, 512, 512, 1024, 8, 2
N_CORES = 8
P = 128
C_DEFAULT = 640  # per-expert token capacity; max routed count is 545 for the fixed seed

LAST_RESULTS = None  # BassKernelResults of the most recent device run (for test.py)

_nc_cache: dict[tuple, object] = {}


def _chunks(C: int) -> list[tuple[int, int]]:
    """Split the free dim C into matmul chunks <=512, preferring >=256 so
    fp32r runs at full rate."""
    n = math.ceil(C / 512)
    base = C // n
    rem = C - base * n
    out = []
    off = 0
    for i in range(n):
        sz = base + (1 if i < rem else 0)
        out.append((off, sz))
        off += sz
    return out


def _build(C: int, dtype_mode: str = "f32r"):
    import concourse.bass as bass
    import concourse.mybir as mybir
    import concourse.tile as tile
    import concourse.tile_sem_assignment as tsa

    # Single completion semaphore for the (single, FIFO) SWDGE queue: keeps
    # every DMA consumer at one wait and the kernel-tail drain within the
    # walrus sync-wait capacity.
    tsa.NUM_SWDGE_GLOBAL_SEMS = 1

    f32 = mybir.dt.float32
    # matmul operand dtype: float32r runs the PE at full rate (vs 4 cyc/row
    # for float32); producers of fp32r-matmul operands must also be fp32r.
    mm_dt = {"f32r": mybir.dt.float32r, "f32": f32}[dtype_mode]

    KD = D // P  # 4  k-tiles for layer 1 (contraction over D)
    KH = H // P  # 8  k-tiles for layer 2 (contraction over H)

    nc = bass.Bass("TRN2", target_bir_lowering=False, debug=False, num_devices=N_CORES)
    xeT_d = nc.dram_tensor("xeT", [D, C], mm_dt, kind="ExternalInput")
    w1_d = nc.dram_tensor("w1", [D, H], mm_dt, kind="ExternalInput")
    w2_d = nc.dram_tensor("w2", [H, D], mm_dt, kind="ExternalInput")
    b12_d = nc.dram_tensor("b12c", [P, KH + KD], f32, kind="ExternalInput")
    out_d = nc.dram_tensor("outT", [D, C], f32, kind="ExternalOutput")

    relu = mybir.ActivationFunctionType.Relu
    ident = mybir.ActivationFunctionType.Identity
    cspans = _chunks(C)

    with tile.TileContext(nc) as tc:
        with (
            tc.tile_pool(name="sb", bufs=1) as sb,
            tc.tile_pool(name="ps", bufs=6, space="PSUM") as ps,
            tc.tile_pool(name="dps", bufs=1, space="PSUM") as dps,
        ):
            w1_t = sb.tile([P, KD, H], mm_dt)
            xe_t = sb.tile([P, KD, C], mm_dt)
            w2_t = sb.tile([P, KH, D], mm_dt)
            b12_t = sb.tile([P, KH + KD], f32)
            hT_t = [sb.tile([P, C], mm_dt, tag=f"h_{k}", name=f"h_{k}") for k in range(KH)]
            out_t = sb.tile([P, KD, C], f32)

            # all DMAs ride the single SWDGE queue: one completion semaphore,
            # FIFO order, so every consumer needs at most one DMA wait and the
            # kernel-tail drain sees a single DMA semaphore.
            nc.gpsimd.dma_start(w1_t[:], w1_d.ap().rearrange("(ko p) h -> p ko h", p=P))
            nc.gpsimd.dma_start(xe_t[:], xeT_d.ap().rearrange("(ko p) c -> p ko c", p=P))
            nc.gpsimd.dma_start(b12_t[:], b12_d.ap())
            nc.gpsimd.dma_start(w2_t[:], w2_d.ap().rearrange("(ko p) d -> p ko d", p=P))

            # Absorbers: soak each DMA-completion wait into a throwaway op so
            # the real matmuls/activations carry at most one wait each.
            dummy_ps = dps.tile([1, 2], f32, tag="dummy", name="dummy")

            def absorb(slab):
                nc.tensor.matmul(
                    dummy_ps[:], slab[:, 0:1], slab[:, 0:2], start=True, stop=True,
                    skip_group_check=True,
                )

            absorb(w1_t[:, 0])
            absorb(xe_t[:, 0])
            dummy_sb = sb.tile([P, 1], f32, name="dummy_sb")
            nc.scalar.copy(dummy_sb[:], b12_t[:, 0:1])

            # layer 1: hT[hi] = relu(sum_k W1[k,hi]^T @ xeT[k] + b1[hi])
            for hi in range(KH):
                for c0, cw in cspans:
                    pt = ps.tile([P, cw], f32, tag="ps")
                    for k in range(KD):
                        nc.tensor.matmul(
                            pt[:],
                            w1_t[:, k, hi * P : (hi + 1) * P],
                            xe_t[:, k, c0 : c0 + cw],
                            start=(k == 0),
                            stop=(k == KD - 1),
                        )
                    nc.scalar.activation(
                        hT_t[hi][:, c0 : c0 + cw], pt[:], relu, bias=b12_t[:, hi : hi + 1]
                    )

            absorb(w2_t[:, 0])

            # layer 2: outT[di] = sum_k W2[k,di]^T @ hT[k] + b2[di]
            for di in range(KD):
                for c0, cw in cspans:
                    pt = ps.tile([P, cw], f32, tag="ps")
                    for k in range(KH):
                        nc.tensor.matmul(
                            pt[:],
                            w2_t[:, k, di * P : (di + 1) * P],
                            hT_t[k][:, c0 : c0 + cw],
                            start=(k == 0),
                            stop=(k == KH - 1),
                        )
                    nc.scalar.activation(
                        out_t[:, di, c0 : c0 + cw],
                        pt[:],
                        ident,
                        bias=b12_t[:, KH + di : KH + di + 1],
                    )

            # single store via HWDGE (fresh lane -> only the ACT data wait)
            nc.sync.dma_start(
                out_d.ap().rearrange("(ko p) c -> p ko c", p=P), out_t[:]
            )

    _split_multi_waits(nc, mybir)
    return nc


def _split_multi_waits(nc, mybir):
    """This walrus build allows only one sync-wait per engine instruction.
    Split any multi-wait instruction: hoist all but the last wait onto
    single-wait NoOps inserted just before it on the same engine."""
    n = 0
    for f in nc.m.functions:
        for blk in f.blocks:
            insts = blk.instructions
            i = 0
            while i < len(insts):
                inst = insts[i]
                si = inst.sync_info
                if si is not None and len(si.on_wait) > 1:
                    waits = list(si.on_wait)
                    for j, w in enumerate(waits[:-1]):
                        nop = mybir.InstNoOp(
                            name=f"ant-waitsplit-{n}",
                            engine=inst.engine,
                            ins=[],
                            outs=[],
                            sync_info=mybir.SyncInfo(on_wait=[w], on_update=[]),
                        )
                        n += 1
                        insts.insert(i, nop)
                        i += 1
                    inst.sync_info = mybir.SyncInfo(
                        on_wait=[waits[-1]], on_update=list(si.on_update)
                    )
                i += 1


def _get_nc(C: int, dtype_mode: str):
    key = (C, dtype_mode)
    if key not in _nc_cache:
        _nc_cache[key] = _build(C, dtype_mode)
    return _nc_cache[key]


def _route(xf: np.ndarray, mask_f: np.ndarray, w_gate: np.ndarray):
    """Top-2 gating on host. Returns per-expert (positions, gate values)."""
    N = xf.shape[0]
    logits = xf @ w_gate  # [N, E] f32
    rows = np.arange(N)
    i1 = np.argmax(logits, axis=1)
    v1 = logits[rows, i1]
    l2 = logits.copy()
    l2[rows, i1] = -np.inf
    i2 = np.argmax(l2, axis=1)
    v2 = l2[rows, i2]
    # softmax over the two top values (v1 >= v2)
    e2 = np.exp(v2 - v1)
    s = 1.0 + e2
    g1 = (1.0 / s).astype(np.float32)
    g2 = (e2 / s).astype(np.float32)
    active = mask_f != 0
    pos, gv = [], []
    for e in range(E):
        s1 = (i1 == e) & active
        s2 = (i2 == e) & active
        p = np.concatenate([np.nonzero(s1)[0], np.nonzero(s2)[0]])
        g = np.concatenate([g1[s1], g2[s2]])
        pos.append(p)
        gv.append(g)
    return pos, gv


def kernel(x, mask, w_gate, W1, b1, W2, b2, dtype_mode="f32r", trace=False):
    global LAST_RESULTS
    from concourse.bass_utils import run_bass_kernel_spmd

    x = np.asarray(x, dtype=np.float32)
    mask_f = np.asarray(mask).reshape(-1)
    w_gate = np.asarray(w_gate, dtype=np.float32)
    W1 = np.asarray(W1, dtype=np.float32)
    b1 = np.asarray(b1, dtype=np.float32)
    W2 = np.asarray(W2, dtype=np.float32)
    b2 = np.asarray(b2, dtype=np.float32)

    xf = x.reshape(-1, D)
    pos, gv = _route(xf, mask_f, w_gate)
    maxc = max(len(p) for p in pos)
    C = max(C_DEFAULT, ((maxc + P - 1) // P) * P)

    nc = _get_nc(C, dtype_mode)

    in_maps = []
    for e in range(E):
        xeT = np.zeros((D, C), dtype=np.float32)
        n_e = len(pos[e])
        if n_e:
            xeT[:, :n_e] = xf[pos[e]].T
        b12 = np.concatenate(
            [b1[e].reshape(H // P, P).T, b2[e].reshape(D // P, P).T], axis=1
        )
        in_maps.append(
            {
                "xeT": xeT,
                "w1": np.ascontiguousarray(W1[e]),
                "w2": np.ascontiguousarray(W2[e]),
                "b12c": np.ascontiguousarray(b12),
            }
        )

    res = run_bass_kernel_spmd(nc, in_maps, core_ids=list(range(N_CORES)), trace=trace)
    LAST_RESULTS = res

    y = xf.copy()
    for e in range(E):
        n_e = len(pos[e])
        if n_e:
            y[pos[e]] += gv[e][:, None] * res.results[e]["outT"][:, :n_e].T
    return y.reshape(B, S, D)


# revision 20
# speedup vs baseline: 1.4462x; 1.4462x over previous
"""MoE top-2 (Switch-style) expert-parallel kernel for Trainium2, 8 NeuronCores.

Model dims (hardcoded from the problem spec):
  x:[B=8,S=512,D=512], w_gate:[D,E=8], W1:[E,D,H=1024], b1:[E,H],
  W2:[E,H,D], b2:[E,D], top-k K=2, mask:[B,S] in {0,1}.

Strategy (expert-parallel, matching the sharding hint):
  - Host computes the gating (logits -> top-2 -> softmax -> dense gates,
    masked) and dispatches: for each expert e, gather the tokens with a
    non-zero gate for e into a capacity-C buffer, transposed to [D, C]
    so the device never has to transpose activations.
  - Each of the 8 cores runs one expert's 2-layer MLP on its [D, C]
    token block, entirely in "transposed" layout:
        hT[H,C]   = relu(W1e^T-form matmul: lhsT=W1e[D,H], rhs=xeT[D,C]) + b1
        outT[D,C] = (lhsT=W2e[H,D], rhs=hT[H,C]) + b2
    fp32 data, fp32r matmuls (full PE rate at N>=256).
  - Host combines: y = x + sum_e gate_e * outT_e^T scattered back to the
    token positions (exactly equal to the dense reference formulation,
    since non-top-2 gates are exactly zero).

Codegen quirk this kernel works around: the walrus build here allows only
ONE sync-wait on Matmult/Activation/DMA instruction structs and ~8 on the
kernel-tail Drain. Hence: one DMA per input tensor (few HWDGE lanes),
tiny "absorber" ops that soak up DMA-completion waits, and the store on
the SWDGE (gpsimd) queue.
"""

import math

import numpy as np

B, S, D, H, E, TOPK = 8, 512, 512, 1024, 8, 2
N_CORES = 8
P = 128
C_DEFAULT = 640  # per-expert token capacity; max routed count is 545 for the fixed seed

LAST_RESULTS = None  # BassKernelResults of the most recent device run (for test.py)

_nc_cache: dict[tuple, object] = {}


def _chunks(C: int) -> list[tuple[int, int]]:
    """Split the free dim C into matmul chunks <=512, preferring >=256 so
    fp32r runs at full rate."""
    n = math.ceil(C / 512)
    base = C // n
    rem = C - base * n
    out = []
    off = 0
    for i in range(n):
        sz = base + (1 if i < rem else 0)
        out.append((off, sz))
        off += sz
    return out


def _build(C: int, dtype_mode: str = "f32r"):
    import concourse.bass as bass
    import concourse.mybir as mybir
    import concourse.tile as tile

    f32 = mybir.dt.float32
    # matmul operand dtype: float32r runs the PE at full rate (vs 4 cyc/row
    # for float32); producers of fp32r-matmul operands must also be fp32r.
    mm_dt = {"f32r": mybir.dt.float32r, "f32": f32}[dtype_mode]

    KD = D // P  # 4  k-tiles for layer 1 (contraction over D)
    KH = H // P  # 8  k-tiles for layer 2 (contraction over H)

    nc = bass.Bass("TRN2", target_bir_lowering=False, debug=False, num_devices=N_CORES)
    xeT_d = nc.dram_tensor("xeT", [D, C], mm_dt, kind="ExternalInput")
    w1_d = nc.dram_tensor("w1", [D, H], mm_dt, kind="ExternalInput")
    w2_d = nc.dram_tensor("w2", [H, D], mm_dt, kind="ExternalInput")
    b12_d = nc.dram_tensor("b12c", [P, KH + KD], f32, kind="ExternalInput")
    out_d = nc.dram_tensor("outT", [D, C], f32, kind="ExternalOutput")

    relu = mybir.ActivationFunctionType.Relu
    ident = mybir.ActivationFunctionType.Identity
    cspans = _chunks(C)

    with tile.TileContext(nc) as tc:
        with (
            tc.tile_pool(name="sb", bufs=1) as sb,
            tc.tile_pool(name="ps", bufs=8, space="PSUM") as ps,
        ):
            w1_t = [sb.tile([P, H], mm_dt, tag=f"w1_{k}", name=f"w1_{k}") for k in range(KD)]
            xe_t = [sb.tile([P, C], mm_dt, tag=f"xe_{k}", name=f"xe_{k}") for k in range(KD)]
            w2_t = [sb.tile([P, D], mm_dt, tag=f"w2_{k}", name=f"w2_{k}") for k in range(KH)]
            b12_t = sb.tile([P, KH + KD], f32)
            hT_t = [sb.tile([P, C], mm_dt, tag=f"h_{k}", name=f"h_{k}") for k in range(KH)]
            out_t = [sb.tile([P, C], f32, tag=f"o_{k}", name=f"o_{k}") for k in range(KD)]

            # per-k-slab HWDGE loads: round-robin across all lanes for full
            # HBM bandwidth; layer-1 operands first so its matmuls start early
            w1_r = w1_d.ap().rearrange("(ko p) h -> p ko h", p=P)
            xe_r = xeT_d.ap().rearrange("(ko p) c -> p ko c", p=P)
            w2_r = w2_d.ap().rearrange("(ko p) d -> p ko d", p=P)
            for k in range(KD):
                nc.sync.dma_start(w1_t[k][:], w1_r[:, k])
                nc.sync.dma_start(xe_t[k][:], xe_r[:, k])
            nc.sync.dma_start(b12_t[:], b12_d.ap())
            for k in range(KH):
                nc.sync.dma_start(w2_t[k][:], w2_r[:, k])

            # layer 1: hT[hi] = relu(sum_k W1[k,hi]^T @ xeT[k] + b1[hi])
            for hi in range(KH):
                for c0, cw in cspans:
                    pt = ps.tile([P, cw], f32, tag="ps")
                    for k in range(KD):
                        nc.tensor.matmul(
                            pt[:],
                            w1_t[k][:, hi * P : (hi + 1) * P],
                            xe_t[k][:, c0 : c0 + cw],
                            start=(k == 0),
                            stop=(k == KD - 1),
                        )
                    nc.scalar.activation(
                        hT_t[hi][:, c0 : c0 + cw], pt[:], relu, bias=b12_t[:, hi : hi + 1]
                    )

            # layer 2: outT[di] = sum_k W2[k,di]^T @ hT[k] + b2[di]
            for di in range(KD):
                for c0, cw in cspans:
                    pt = ps.tile([P, cw], f32, tag="ps")
                    for k in range(KH):
                        nc.tensor.matmul(
                            pt[:],
                            w2_t[k][:, di * P : (di + 1) * P],
                            hT_t[k][:, c0 : c0 + cw],
                            start=(k == 0),
                            stop=(k == KH - 1),
                        )
                    nc.scalar.activation(
                        out_t[di][:, c0 : c0 + cw],
                        pt[:],
                        ident,
                        bias=b12_t[:, KH + di : KH + di + 1],
                    )

            # per-slab stores so the first slabs go out while L2 still runs
            o_r = out_d.ap().rearrange("(ko p) c -> p ko c", p=P)
            for k in range(KD):
                nc.sync.dma_start(o_r[:, k], out_t[k][:])

    _split_multi_waits(nc, mybir)
    return nc


def _split_multi_waits(nc, mybir):
    """This walrus build allows only one sync-wait per engine instruction.
    Split any multi-wait instruction: hoist all but the last wait onto
    single-wait NoOps inserted just before it on the same engine."""
    n = 0
    for f in nc.m.functions:
        for blk in f.blocks:
            insts = blk.instructions
            i = 0
            while i < len(insts):
                inst = insts[i]
                si = inst.sync_info
                if si is not None and len(si.on_wait) > 1:
                    waits = list(si.on_wait)
                    for j, w in enumerate(waits[:-1]):
                        nop = mybir.InstNoOp(
                            name=f"ant-waitsplit-{n}",
                            engine=inst.engine,
                            ins=[],
                            outs=[],
                            sync_info=mybir.SyncInfo(on_wait=[w], on_update=[]),
                        )
                        n += 1
                        insts.insert(i, nop)
                        i += 1
                    inst.sync_info = mybir.SyncInfo(
                        on_wait=[waits[-1]], on_update=list(si.on_update)
                    )
                i += 1


def _get_nc(C: int, dtype_mode: str):
    key = (C, dtype_mode)
    if key not in _nc_cache:
        _nc_cache[key] = _build(C, dtype_mode)
    return _nc_cache[key]


def _route(xf: np.ndarray, mask_f: np.ndarray, w_gate: np.ndarray):
    """Top-2 gating on host. Returns per-expert (positions, gate values)."""
    N = xf.shape[0]
    logits = xf @ w_gate  # [N, E] f32
    rows = np.arange(N)
    i1 = np.argmax(logits, axis=1)
    v1 = logits[rows, i1]
    l2 = logits.copy()
    l2[rows, i1] = -np.inf
    i2 = np.argmax(l2, axis=1)
    v2 = l2[rows, i2]
    # softmax over the two top values (v1 >= v2)
    e2 = np.exp(v2 - v1)
    s = 1.0 + e2
    g1 = (1.0 / s).astype(np.float32)
    g2 = (e2 / s).astype(np.float32)
    active = mask_f != 0
    pos, gv = [], []
    for e in range(E):
        s1 = (i1 == e) & active
        s2 = (i2 == e) & active
        p = np.concatenate([np.nonzero(s1)[0], np.nonzero(s2)[0]])
        g = np.concatenate([g1[s1], g2[s2]])
        pos.append(p)
        gv.append(g)
    return pos, gv


def kernel(x, mask, w_gate, W1, b1, W2, b2, dtype_mode="f32r", trace=False):
    global LAST_RESULTS
    from concourse.bass_utils import run_bass_kernel_spmd

    x = np.asarray(x, dtype=np.float32)
    mask_f = np.asarray(mask).reshape(-1)
    w_gate = np.asarray(w_gate, dtype=np.float32)
    W1 = np.asarray(W1, dtype=np.float32)
    b1 = np.asarray(b1, dtype=np.float32)
    W2 = np.asarray(W2, dtype=np.float32)
    b2 = np.asarray(b2, dtype=np.float32)

    xf = x.reshape(-1, D)
    pos, gv = _route(xf, mask_f, w_gate)
    maxc = max(len(p) for p in pos)
    C = max(C_DEFAULT, ((maxc + P - 1) // P) * P)

    nc = _get_nc(C, dtype_mode)

    in_maps = []
    for e in range(E):
        xeT = np.zeros((D, C), dtype=np.float32)
        n_e = len(pos[e])
        if n_e:
            xeT[:, :n_e] = xf[pos[e]].T
        b12 = np.concatenate(
            [b1[e].reshape(H // P, P).T, b2[e].reshape(D // P, P).T], axis=1
        )
        in_maps.append(
            {
                "xeT": xeT,
                "w1": np.ascontiguousarray(W1[e]),
                "w2": np.ascontiguousarray(W2[e]),
                "b12c": np.ascontiguousarray(b12),
            }
        )

    res = run_bass_kernel_spmd(nc, in_maps, core_ids=list(range(N_CORES)), trace=trace)
    LAST_RESULTS = res

    y = xf.copy()
    for e in range(E):
        n_e = len(pos[e])
        if n_e:
            y[pos[e]] += gv[e][:, None] * res.results[e]["outT"][:, :n_e].T
    return y.reshape(B, S, D)
